# revision 2
# baseline (speedup 1.0000x reference)
"""TRN2 Bass kernel for nn_APCriterion (retrieval_knn AP loss).

Math (uniform crops, n_crops=64, k=128, d=128, knn=20, nq=20):
  sim = anc @ pos.T                       [8192, 8192]
  per row r in crop i:  pos value = sim[r, r]
                        negatives = top-20 of sim[r, :] excluding crop i's cols
  AP per row via the R2D2 triangular-bin quantizer, outputs:
  (1 - ap.mean(), ap.mean())

Distribution: rows sharded across 8 cores (1024 rows = 8 crops each), pos
replicated (pre-transposed and per-core rotated on the host so the own-crop
block lands at a core-invariant column offset -> identical SPMD graph).
No collectives; the final mean over 8192 per-row APs happens on the host.

Per-core device pipeline (per 128-row m-tile):
  PE   : 16 f32r matmuls (128x128 @ 128x512) -> PSUM
  ACT  : evacuate PSUM -> SBUF bf16 with scale 19 (t = 19*sim)
  DVE  : diagonal extract (f32, from PSUM, via 19*I mask + reduce),
         own-block memset, fold cascade by halves 8192->256 (bf16 max),
         top-20 via 3x(max8 + match_replace),
         then one AP-histogram stage per core (15 triangular bins,
         negated-hat formulation, cumsums via shifted adds).
"""

import os
import sys

import numpy as np

if "/opt/trn_rl_repo" not in sys.path:
    sys.path.insert(0, "/opt/trn_rl_repo")

N_CORES = 8
B = 8192           # total rows/cols of sim
D = 128            # descriptor dim (= partition/contraction dim)
KNN = 20
ROWS_PER_CORE = B // N_CORES   # 1024
MT = ROWS_PER_CORE // 128      # 8 m-tiles (crops) per core
NB = 15            # histogram bins c=0..14 (t = 19*sim is < 13 for unit vecs w/ margin)
PADW = 24          # cumsum buffer inner width; bins live at [8, 8+NB)
CHUNK = 2048       # PSUM tile free width (4 banks)
NCHUNK = B // CHUNK            # 4 psum tiles per m-tile
FOLD_TO = 256
NEG_FILL = -28672.0            # exactly representable in bf16

_GRAPH = None
_LAST_RESULTS = None


def _build_graph():
    import concourse.bass as bass
    import concourse.tile as tile
    from concourse import bacc, mybir
    from contextlib import ExitStack

    dt = mybir.dt
    Alu = mybir.AluOpType

    nc = bacc.Bacc("TRN2", target_bir_lowering=False, debug=False)

    ancT_d = nc.dram_tensor("ancT", [D, ROWS_PER_CORE], dt.float32r,
                            kind="ExternalInput").ap()
    posT_d = nc.dram_tensor("posT", [D, B], dt.float32r,
                            kind="ExternalInput").ap()
    eye_d = nc.dram_tensor("eye19", [128, 128], dt.float32,
                           kind="ExternalInput").ap()
    out_d = nc.dram_tensor("out", [128, MT], dt.float32,
                           kind="ExternalOutput").ap()

    with tile.TileContext(nc) as tc, ExitStack() as ctx:
        const = ctx.enter_context(tc.tile_pool(name="const", bufs=1))
        psum = ctx.enter_context(tc.tile_pool(name="psum", bufs=2, space="PSUM"))
        tbufp = ctx.enter_context(tc.tile_pool(name="tbuf", bufs=2))
        foldp = ctx.enter_context(tc.tile_pool(name="fold", bufs=2))
        smallp = ctx.enter_context(tc.tile_pool(name="small", bufs=2))

        # ---- persistent tiles ----
        posT = const.tile([D, B], dt.float32r, tag="posT")
        ancT = const.tile([D, ROWS_PER_CORE], dt.float32r, tag="ancT")
        eye19 = const.tile([128, 128], dt.float32, tag="eye19")
        # per-(row, m-tile) t-values: [pos, 20 negs] at stride 32
        t_all = const.tile([128, MT * 32], dt.float32, tag="t_all")
        # AP-stage buffers [128, MT, PADW] flattened
        recC = const.tile([128, MT * PADW], dt.float32, tag="recC")
        nbsC = const.tile([128, MT * PADW], dt.float32, tag="nbsC")
        Xb = const.tile([128, MT * PADW], dt.float32, tag="Xb")
        Yb = const.tile([128, MT * PADW], dt.float32, tag="Yb")
        Zb = const.tile([128, MT * PADW], dt.float32, tag="Zb")
        Ub = const.tile([128, MT * 32], dt.float32, tag="Ub")
        Vb = const.tile([128, MT * 32], dt.float32, tag="Vb")
        Sb = const.tile([128, MT], dt.float32, tag="Sb")

        # ---- input DMAs ----
        for c in range(B // 512):
            nc.sync.dma_start(out=posT[:, c * 512:(c + 1) * 512],
                              in_=posT_d[:, c * 512:(c + 1) * 512])
        nc.sync.dma_start(out=ancT[:], in_=ancT_d[:])
        nc.sync.dma_start(out=eye19[:], in_=eye_d[:])

        # zero-init cumsum buffers (pads must stay zero)
        for buf in (recC, nbsC, Xb, Yb, Zb):
            nc.vector.memset(buf[:], 0.0)

        # ---- per m-tile: matmul -> evac -> fold -> top-20 ----
        for g in range(MT):
            lhsT = ancT[:, g * 128:(g + 1) * 128]
            tbuf = tbufp.tile([128, B], dt.bfloat16, tag="tbuf")
            ps0 = None
            for pc in range(NCHUNK):
                ps = psum.tile([128, CHUNK], dt.float32, tag="ps")
                if pc == 0:
                    ps0 = ps
                for j in range(CHUNK // 512):
                    n0 = j * 512
                    nc.tensor.matmul(ps[:, n0:n0 + 512], lhsT=lhsT,
                                     rhs=posT[:, pc * CHUNK + n0:
                                              pc * CHUNK + n0 + 512],
                                     start=True, stop=True)
                # evacuate to bf16 t-space (t = 19*sim)
                nc.scalar.mul(tbuf[:, pc * CHUNK:(pc + 1) * CHUNK], ps[:], 19.0)
                if pc == 0:
                    # diagonal (positive) in f32: reduce(ps_block * 19I)
                    scr = smallp.tile([128, 128], dt.float32, tag="scr")
                    nc.vector.tensor_tensor(out=scr[:],
                                            in0=ps[:, g * 128:(g + 1) * 128],
                                            in1=eye19[:], op=Alu.mult)
                    nc.vector.tensor_reduce(out=t_all[:, g * 32:g * 32 + 1],
                                            in_=scr[:],
                                            axis=mybir.AxisListType.X,
                                            op=Alu.add)
            # mask own crop block
            nc.vector.memset(tbuf[:, g * 128:(g + 1) * 128], NEG_FILL)
            # fold cascade by halves: 8192 -> 256 (bf16, 2x mode)
            f1 = foldp.tile([128, 4096], dt.bfloat16, tag="f1")
            nc.vector.tensor_tensor(out=f1[:], in0=tbuf[:, :4096],
                                    in1=tbuf[:, 4096:], op=Alu.max)
            f2 = foldp.tile([128, 2048], dt.bfloat16, tag="f2")
            nc.vector.tensor_tensor(out=f2[:], in0=f1[:, :2048],
                                    in1=f1[:, 2048:], op=Alu.max)
            f3 = foldp.tile([128, 1024], dt.bfloat16, tag="f3")
            nc.vector.tensor_tensor(out=f3[:], in0=f2[:, :1024],
                                    in1=f2[:, 1024:], op=Alu.max)
            f4 = foldp.tile([128, 512], dt.bfloat16, tag="f4")
            nc.vector.tensor_tensor(out=f4[:], in0=f3[:, :512],
                                    in1=f3[:, 512:], op=Alu.max)
            f5 = foldp.tile([128, FOLD_TO], dt.bfloat16, tag="f5")
            nc.vector.tensor_tensor(out=f5[:], in0=f4[:, :FOLD_TO],
                                    in1=f4[:, FOLD_TO:], op=Alu.max)
            # top-20 via 3x(max8 + match_replace)
            r1 = smallp.tile([128, 8], dt.bfloat16, tag="r1")
            nc.vector.max(out=r1[:], in_=f5[:])
            w2 = foldp.tile([128, FOLD_TO], dt.bfloat16, tag="w2")
            nc.vector.match_replace(out=w2[:], in_to_replace=r1[:],
                                    in_values=f5[:], imm_value=NEG_FILL)
            r2 = smallp.tile([128, 8], dt.bfloat16, tag="r2")
            nc.vector.max(out=r2[:], in_=w2[:])
            w3 = foldp.tile([128, FOLD_TO], dt.bfloat16, tag="w3")
            nc.vector.match_replace(out=w3[:], in_to_replace=r2[:],
                                    in_values=w2[:], imm_value=NEG_FILL)
            r3 = smallp.tile([128, 8], dt.bfloat16, tag="r3")
            nc.vector.max(out=r3[:], in_=w3[:])
            # write negatives (bf16 -> f32)
            nc.vector.tensor_copy(out=t_all[:, g * 32 + 1:g * 32 + 9], in_=r1[:])
            nc.vector.tensor_copy(out=t_all[:, g * 32 + 9:g * 32 + 17], in_=r2[:])
            nc.vector.tensor_copy(out=t_all[:, g * 32 + 17:g * 32 + 21],
                                  in_=r3[:, 0:4])

        # ---- AP histogram stage (once per core) ----
        t3 = t_all[:].rearrange("p (g w) -> p g w", w=32)[:, :, 0:21]
        u3 = Ub[:].rearrange("p (g w) -> p g w", w=32)[:, :, 0:21]
        v3 = Vb[:].rearrange("p (g w) -> p g w", w=32)[:, :, 0:21]
        rec3 = recC[:].rearrange("p (g w) -> p g w", w=PADW)
        nbs3 = nbsC[:].rearrange("p (g w) -> p g w", w=PADW)
        X3 = Xb[:].rearrange("p (g w) -> p g w", w=PADW)
        Y3 = Yb[:].rearrange("p (g w) -> p g w", w=PADW)
        Z3 = Zb[:].rearrange("p (g w) -> p g w", w=PADW)

        for b in range(NB):
            c = NB - 1 - b
            if c == 0:
                # v = min(max(t,0) - 1, 0)  (= -q for the left-saturated bin)
                nc.vector.tensor_scalar(u3, t3, 0.0, 1.0,
                                        op0=Alu.max, op1=Alu.subtract)
                nc.vector.tensor_scalar(v3, u3, 0.0, None, op0=Alu.min)
            else:
                # v = min(|t - c| - 1, 0)   (= -hat_c(t));  |t-c| built as
                # max(t-c, c-t) since abs_max is not HW-encodable here.
                nc.vector.tensor_scalar(u3, t3, -1.0, float(c),
                                        op0=Alu.mult, op1=Alu.add)  # c - t
                nc.vector.scalar_tensor_tensor(out=v3, in0=t3, scalar=float(c),
                                               in1=u3, op0=Alu.subtract,
                                               op1=Alu.max)         # |t - c|
                nc.vector.tensor_scalar(v3, v3, 1.0, 0.0,
                                        op0=Alu.subtract, op1=Alu.min)
            nc.vector.tensor_reduce(out=nbs3[:, :, 8 + b:9 + b], in_=v3,
                                    axis=mybir.AxisListType.X, op=Alu.add)
            nc.vector.tensor_copy(out=rec3[:, :, 8 + b:9 + b],
                                  in_=v3[:, :, 0:1])

        lo, hi = 8, 8 + NB
        # cumsum over bins (shifted adds; pads are zero)
        nc.vector.tensor_tensor(out=X3[:, :, lo:hi], in0=rec3[:, :, lo:hi],
                                in1=rec3[:, :, lo - 1:hi - 1], op=Alu.add)
        nc.vector.tensor_tensor(out=Y3[:, :, lo:hi], in0=X3[:, :, lo:hi],
                                in1=X3[:, :, lo - 2:hi - 2], op=Alu.add)
        nc.vector.tensor_tensor(out=X3[:, :, lo:hi], in0=Y3[:, :, lo:hi],
                                in1=Y3[:, :, lo - 4:hi - 4], op=Alu.add)
        nc.vector.tensor_tensor(out=Y3[:, :, lo:hi], in0=X3[:, :, lo:hi],
                                in1=X3[:, :, lo - 8:hi - 8], op=Alu.add)
        # -> crec in Y
        nc.vector.tensor_tensor(out=X3[:, :, lo:hi], in0=nbs3[:, :, lo:hi],
                                in1=nbs3[:, :, lo - 1:hi - 1], op=Alu.add)
        nc.vector.tensor_tensor(out=Z3[:, :, lo:hi], in0=X3[:, :, lo:hi],
                                in1=X3[:, :, lo - 2:hi - 2], op=Alu.add)
        nc.vector.tensor_tensor(out=X3[:, :, lo:hi], in0=Z3[:, :, lo:hi],
                                in1=Z3[:, :, lo - 4:hi - 4], op=Alu.add)
        nc.vector.tensor_tensor(out=Z3[:, :, lo:hi], in0=X3[:, :, lo:hi],
                                in1=X3[:, :, lo - 8:hi - 8], op=Alu.add)
        # -> cnbs in Z
        # prec = crec / (cnbs - 1e-16)  (both stored negated -> prec >= 0)
        nc.vector.tensor_scalar(X3[:, :, lo:hi], Z3[:, :, lo:hi], 1e-16, None,
                                op0=Alu.subtract)
        nc.vector.reciprocal(out=nbs3[:, :, lo:hi], in_=X3[:, :, lo:hi])
        nc.vector.tensor_tensor(out=X3[:, :, lo:hi], in0=Y3[:, :, lo:hi],
                                in1=nbs3[:, :, lo:hi], op=Alu.mult)
        # S = sum_b prec * rec_neg  (= -ap)
        nc.vector.tensor_tensor(out=Y3[:, :, lo:hi], in0=X3[:, :, lo:hi],
                                in1=rec3[:, :, lo:hi], op=Alu.mult)
        nc.vector.tensor_reduce(out=Sb[:], in_=Y3[:, :, lo:hi],
                                axis=mybir.AxisListType.X, op=Alu.add)
        nc.sync.dma_start(out=out_d[:], in_=Sb[:])

    nc.compile()
    return nc


def _get_graph():
    global _GRAPH
    if _GRAPH is None:
        _GRAPH = _build_graph()
    return _GRAPH


def kernel(anc_feat, pos_feat, kpts_crop_ids):
    from concourse.bass_utils import run_bass_kernel_spmd

    global _LAST_RESULTS
    anc = np.ascontiguousarray(np.asarray(anc_feat, dtype=np.float32))
    pos = np.ascontiguousarray(np.asarray(pos_feat, dtype=np.float32))
    assert anc.shape == (B, D) and pos.shape == (B, D)

    nc = _get_graph()

    posT = np.ascontiguousarray(pos.T)                 # [128, 8192]
    eye19 = (np.eye(128) * 19.0).astype(np.float32)
    in_maps = []
    for c in range(N_CORES):
        ancT_c = np.ascontiguousarray(
            anc[c * ROWS_PER_CORE:(c + 1) * ROWS_PER_CORE].T)
        posT_c = np.ascontiguousarray(np.roll(posT, -c * ROWS_PER_CORE, axis=1))
        in_maps.append({"ancT": ancT_c, "posT": posT_c, "eye19": eye19})

    trace = os.environ.get("APC_TRACE", "0") == "1"
    res = run_bass_kernel_spmd(nc, in_maps, core_ids=list(range(N_CORES)),
                               trace=trace)
    _LAST_RESULTS = res

    S = np.stack([np.asarray(res.results[c]["out"]) for c in range(N_CORES)])
    # S[c][p, g] = -ap(row c*1024 + g*128 + p)
    ap = -S.transpose(0, 2, 1).reshape(-1).astype(np.float64)
    apm = ap.mean()
    return np.array([1.0 - apm, apm], dtype=np.float32)


# revision 3
# speedup vs baseline: 173.3660x; 173.3660x over previous
"""TRN2 Bass kernel for nn_APCriterion (retrieval_knn AP loss).

Math (uniform crops, n_crops=64, k=128, d=128, knn=20, nq=20):
  sim = anc @ pos.T                       [8192, 8192]
  per row r in crop i:  pos value = sim[r, r]
                        negatives = top-20 of sim[r, :] excluding crop i's cols
  AP per row via the R2D2 triangular-bin quantizer, outputs:
  (1 - ap.mean(), ap.mean())

Distribution: rows sharded across 8 cores (1024 rows = 8 crops each), pos
replicated (pre-transposed and per-core rotated on the host so the own-crop
block lands at a core-invariant column offset -> identical SPMD graph).
No collectives; the final mean over 8192 per-row APs happens on the host.

Per-core device pipeline (per 128-row m-tile):
  PE   : 16 f32r matmuls (128x128 @ 128x512) -> PSUM
  ACT  : evacuate PSUM -> SBUF bf16 with scale 19 (t = 19*sim)
  DVE  : diagonal extract (f32, from PSUM, via 19*I mask + reduce),
         own-block memset, fold cascade by halves 8192->256 (bf16 max),
         top-20 via 3x(max8 + match_replace),
         then one AP-histogram stage per core (15 triangular bins,
         negated-hat formulation, cumsums via shifted adds).
"""

import os
import sys

import numpy as np

if "/opt/trn_rl_repo" not in sys.path:
    sys.path.insert(0, "/opt/trn_rl_repo")

N_CORES = 8
B = 8192           # total rows/cols of sim
D = 128            # descriptor dim (= partition/contraction dim)
KNN = 20
ROWS_PER_CORE = B // N_CORES   # 1024
MT = ROWS_PER_CORE // 128      # 8 m-tiles (crops) per core
NB = 15            # histogram bins c=0..14 (t = 19*sim is < 13 for unit vecs w/ margin)
PADW = 24          # cumsum buffer inner width; bins live at [8, 8+NB)
CHUNK = 2048       # PSUM tile free width (4 banks)
NCHUNK = B // CHUNK            # 4 psum tiles per m-tile
FOLD_TO = 256
NEG_FILL = -28672.0            # exactly representable in bf16

_GRAPH = None
_LAST_RESULTS = None


def _build_graph(loop_repeat=1):
    import concourse.bass as bass
    import concourse.tile as tile
    from concourse import bacc, mybir
    from contextlib import ExitStack

    dt = mybir.dt
    Alu = mybir.AluOpType

    nc = bacc.Bacc("TRN2", target_bir_lowering=False, debug=False)

    ancT_d = nc.dram_tensor("ancT", [D, ROWS_PER_CORE], dt.float32r,
                            kind="ExternalInput").ap()
    posT_d = nc.dram_tensor("posT", [D, B], dt.float32r,
                            kind="ExternalInput").ap()
    eye_d = nc.dram_tensor("eye19", [128, 128], dt.float32,
                           kind="ExternalInput").ap()
    out_d = nc.dram_tensor("out", [128, MT], dt.float32,
                           kind="ExternalOutput").ap()

    with tile.TileContext(nc) as tc, ExitStack() as ctx:
        const = ctx.enter_context(tc.tile_pool(name="const", bufs=1))
        psum = ctx.enter_context(tc.tile_pool(name="psum", bufs=2, space="PSUM"))
        tbufp = ctx.enter_context(tc.tile_pool(name="tbuf", bufs=2))
        foldp = ctx.enter_context(tc.tile_pool(name="fold", bufs=2))
        smallp = ctx.enter_context(tc.tile_pool(name="small", bufs=2))

        # ---- persistent tiles ----
        posT = const.tile([D, B], dt.float32r, tag="posT")
        ancT = const.tile([D, ROWS_PER_CORE], dt.float32r, tag="ancT")
        eye19 = const.tile([128, 128], dt.float32, tag="eye19")
        # per-(row, m-tile) t-values: [pos, 20 negs] at stride 32
        t_all = const.tile([128, MT * 32], dt.float32, tag="t_all")
        # AP-stage buffers [128, MT, PADW] flattened
        recC = const.tile([128, MT * PADW], dt.float32, tag="recC")
        nbsC = const.tile([128, MT * PADW], dt.float32, tag="nbsC")
        Xb = const.tile([128, MT * PADW], dt.float32, tag="Xb")
        Yb = const.tile([128, MT * PADW], dt.float32, tag="Yb")
        Zb = const.tile([128, MT * PADW], dt.float32, tag="Zb")
        Ub = const.tile([128, MT * 32], dt.float32, tag="Ub")
        Vb = const.tile([128, MT * 32], dt.float32, tag="Vb")
        Sb = const.tile([128, MT], dt.float32, tag="Sb")

        # ---- input DMAs ----
        for c in range(B // 512):
            nc.sync.dma_start(out=posT[:, c * 512:(c + 1) * 512],
                              in_=posT_d[:, c * 512:(c + 1) * 512])
        nc.sync.dma_start(out=ancT[:], in_=ancT_d[:])
        nc.sync.dma_start(out=eye19[:], in_=eye_d[:])

        # zero-init cumsum buffers (pads must stay zero)
        for buf in (recC, nbsC, Xb, Yb, Zb):
            nc.vector.memset(buf[:], 0.0)

        def emit_body():
            _emit_core_body(nc, tc, mybir, Alu, dt,
                            posT, ancT, eye19, t_all, recC, nbsC, Xb, Yb, Zb,
                            Ub, Vb, Sb, psum, tbufp, foldp, smallp, out_d)

        if loop_repeat == 1:
            emit_body()
        else:
            with tc.For_i(0, loop_repeat, 1):
                emit_body()

    nc.compile()
    return nc


def _emit_core_body(nc, tc, mybir, Alu, dt,
                    posT, ancT, eye19, t_all, recC, nbsC, Xb, Yb, Zb,
                    Ub, Vb, Sb, psum, tbufp, foldp, smallp, out_d):
    if True:
        # ---- per m-tile: matmul -> evac -> fold -> top-20 ----
        for g in range(MT):
            lhsT = ancT[:, g * 128:(g + 1) * 128]
            tbuf = tbufp.tile([128, B], dt.bfloat16, tag="tbuf")
            ps0 = None
            for pc in range(NCHUNK):
                ps = psum.tile([128, CHUNK], dt.float32, tag="ps")
                if pc == 0:
                    ps0 = ps
                for j in range(CHUNK // 512):
                    n0 = j * 512
                    nc.tensor.matmul(ps[:, n0:n0 + 512], lhsT=lhsT,
                                     rhs=posT[:, pc * CHUNK + n0:
                                              pc * CHUNK + n0 + 512],
                                     start=True, stop=True)
                # evacuate to bf16 t-space (t = 19*sim)
                nc.scalar.mul(tbuf[:, pc * CHUNK:(pc + 1) * CHUNK], ps[:], 19.0)
                if pc == 0:
                    # diagonal (positive) in f32: reduce(ps_block * 19I)
                    scr = smallp.tile([128, 128], dt.float32, tag="scr")
                    nc.vector.tensor_tensor(out=scr[:],
                                            in0=ps[:, g * 128:(g + 1) * 128],
                                            in1=eye19[:], op=Alu.mult)
                    nc.vector.tensor_reduce(out=t_all[:, g * 32:g * 32 + 1],
                                            in_=scr[:],
                                            axis=mybir.AxisListType.X,
                                            op=Alu.add)
            # mask own crop block
            nc.vector.memset(tbuf[:, g * 128:(g + 1) * 128], NEG_FILL)
            # fold cascade by halves: 8192 -> 256 (bf16, 2x mode)
            f1 = foldp.tile([128, 4096], dt.bfloat16, tag="f1")
            nc.vector.tensor_tensor(out=f1[:], in0=tbuf[:, :4096],
                                    in1=tbuf[:, 4096:], op=Alu.max)
            f2 = foldp.tile([128, 2048], dt.bfloat16, tag="f2")
            nc.vector.tensor_tensor(out=f2[:], in0=f1[:, :2048],
                                    in1=f1[:, 2048:], op=Alu.max)
            f3 = foldp.tile([128, 1024], dt.bfloat16, tag="f3")
            nc.vector.tensor_tensor(out=f3[:], in0=f2[:, :1024],
                                    in1=f2[:, 1024:], op=Alu.max)
            f4 = foldp.tile([128, 512], dt.bfloat16, tag="f4")
            nc.vector.tensor_tensor(out=f4[:], in0=f3[:, :512],
                                    in1=f3[:, 512:], op=Alu.max)
            f5 = foldp.tile([128, FOLD_TO], dt.bfloat16, tag="f5")
            nc.vector.tensor_tensor(out=f5[:], in0=f4[:, :FOLD_TO],
                                    in1=f4[:, FOLD_TO:], op=Alu.max)
            # top-20 via 3x(max8 + match_replace)
            r1 = smallp.tile([128, 8], dt.bfloat16, tag="r1")
            nc.vector.max(out=r1[:], in_=f5[:])
            w2 = foldp.tile([128, FOLD_TO], dt.bfloat16, tag="w2")
            nc.vector.match_replace(out=w2[:], in_to_replace=r1[:],
                                    in_values=f5[:], imm_value=NEG_FILL)
            r2 = smallp.tile([128, 8], dt.bfloat16, tag="r2")
            nc.vector.max(out=r2[:], in_=w2[:])
            w3 = foldp.tile([128, FOLD_TO], dt.bfloat16, tag="w3")
            nc.vector.match_replace(out=w3[:], in_to_replace=r2[:],
                                    in_values=w2[:], imm_value=NEG_FILL)
            r3 = smallp.tile([128, 8], dt.bfloat16, tag="r3")
            nc.vector.max(out=r3[:], in_=w3[:])
            # write negatives (bf16 -> f32)
            nc.vector.tensor_copy(out=t_all[:, g * 32 + 1:g * 32 + 9], in_=r1[:])
            nc.vector.tensor_copy(out=t_all[:, g * 32 + 9:g * 32 + 17], in_=r2[:])
            nc.vector.tensor_copy(out=t_all[:, g * 32 + 17:g * 32 + 21],
                                  in_=r3[:, 0:4])

        # ---- AP histogram stage (once per core) ----
        t3 = t_all[:].rearrange("p (g w) -> p g w", w=32)[:, :, 0:21]
        u3 = Ub[:].rearrange("p (g w) -> p g w", w=32)[:, :, 0:21]
        v3 = Vb[:].rearrange("p (g w) -> p g w", w=32)[:, :, 0:21]
        rec3 = recC[:].rearrange("p (g w) -> p g w", w=PADW)
        nbs3 = nbsC[:].rearrange("p (g w) -> p g w", w=PADW)
        X3 = Xb[:].rearrange("p (g w) -> p g w", w=PADW)
        Y3 = Yb[:].rearrange("p (g w) -> p g w", w=PADW)
        Z3 = Zb[:].rearrange("p (g w) -> p g w", w=PADW)

        for b in range(NB):
            c = NB - 1 - b
            if c == 0:
                # v = min(max(t,0) - 1, 0)  (= -q for the left-saturated bin)
                nc.vector.tensor_scalar(u3, t3, 0.0, 1.0,
                                        op0=Alu.max, op1=Alu.subtract)
                nc.vector.tensor_scalar(v3, u3, 0.0, None, op0=Alu.min)
            else:
                # v = min(|t - c| - 1, 0)   (= -hat_c(t));  |t-c| built as
                # max(t-c, c-t) since abs_max is not HW-encodable here.
                nc.vector.tensor_scalar(u3, t3, -1.0, float(c),
                                        op0=Alu.mult, op1=Alu.add)  # c - t
                nc.vector.scalar_tensor_tensor(out=v3, in0=t3, scalar=float(c),
                                               in1=u3, op0=Alu.subtract,
                                               op1=Alu.max)         # |t - c|
                nc.vector.tensor_scalar(v3, v3, 1.0, 0.0,
                                        op0=Alu.subtract, op1=Alu.min)
            nc.vector.tensor_reduce(out=nbs3[:, :, 8 + b:9 + b], in_=v3,
                                    axis=mybir.AxisListType.X, op=Alu.add)
            nc.vector.tensor_copy(out=rec3[:, :, 8 + b:9 + b],
                                  in_=v3[:, :, 0:1])

        lo, hi = 8, 8 + NB
        # cumsum over bins (shifted adds; pads are zero)
        nc.vector.tensor_tensor(out=X3[:, :, lo:hi], in0=rec3[:, :, lo:hi],
                                in1=rec3[:, :, lo - 1:hi - 1], op=Alu.add)
        nc.vector.tensor_tensor(out=Y3[:, :, lo:hi], in0=X3[:, :, lo:hi],
                                in1=X3[:, :, lo - 2:hi - 2], op=Alu.add)
        nc.vector.tensor_tensor(out=X3[:, :, lo:hi], in0=Y3[:, :, lo:hi],
                                in1=Y3[:, :, lo - 4:hi - 4], op=Alu.add)
        nc.vector.tensor_tensor(out=Y3[:, :, lo:hi], in0=X3[:, :, lo:hi],
                                in1=X3[:, :, lo - 8:hi - 8], op=Alu.add)
        # -> crec in Y
        nc.vector.tensor_tensor(out=X3[:, :, lo:hi], in0=nbs3[:, :, lo:hi],
                                in1=nbs3[:, :, lo - 1:hi - 1], op=Alu.add)
        nc.vector.tensor_tensor(out=Z3[:, :, lo:hi], in0=X3[:, :, lo:hi],
                                in1=X3[:, :, lo - 2:hi - 2], op=Alu.add)
        nc.vector.tensor_tensor(out=X3[:, :, lo:hi], in0=Z3[:, :, lo:hi],
                                in1=Z3[:, :, lo - 4:hi - 4], op=Alu.add)
        nc.vector.tensor_tensor(out=Z3[:, :, lo:hi], in0=X3[:, :, lo:hi],
                                in1=X3[:, :, lo - 8:hi - 8], op=Alu.add)
        # -> cnbs in Z
        # prec = crec / (cnbs - 1e-16)  (both stored negated -> prec >= 0)
        nc.vector.tensor_scalar(X3[:, :, lo:hi], Z3[:, :, lo:hi], 1e-16, None,
                                op0=Alu.subtract)
        nc.vector.reciprocal(out=nbs3[:, :, lo:hi], in_=X3[:, :, lo:hi])
        nc.vector.tensor_tensor(out=X3[:, :, lo:hi], in0=Y3[:, :, lo:hi],
                                in1=nbs3[:, :, lo:hi], op=Alu.mult)
        # S = sum_b prec * rec_neg  (= -ap)
        nc.vector.tensor_tensor(out=Y3[:, :, lo:hi], in0=X3[:, :, lo:hi],
                                in1=rec3[:, :, lo:hi], op=Alu.mult)
        nc.vector.tensor_reduce(out=Sb[:], in_=Y3[:, :, lo:hi],
                                axis=mybir.AxisListType.X, op=Alu.add)
        nc.sync.dma_start(out=out_d[:], in_=Sb[:])


def _get_graph():
    global _GRAPH
    if _GRAPH is None:
        _GRAPH = _build_graph()
    return _GRAPH


def kernel(anc_feat, pos_feat, kpts_crop_ids):
    from concourse.bass_utils import run_bass_kernel_spmd

    global _LAST_RESULTS
    anc = np.ascontiguousarray(np.asarray(anc_feat, dtype=np.float32))
    pos = np.ascontiguousarray(np.asarray(pos_feat, dtype=np.float32))
    assert anc.shape == (B, D) and pos.shape == (B, D)

    nc = _get_graph()

    posT = np.ascontiguousarray(pos.T)                 # [128, 8192]
    eye19 = (np.eye(128) * 19.0).astype(np.float32)
    in_maps = []
    for c in range(N_CORES):
        ancT_c = np.ascontiguousarray(
            anc[c * ROWS_PER_CORE:(c + 1) * ROWS_PER_CORE].T)
        posT_c = np.ascontiguousarray(np.roll(posT, -c * ROWS_PER_CORE, axis=1))
        in_maps.append({"ancT": ancT_c, "posT": posT_c, "eye19": eye19})

    trace = os.environ.get("APC_TRACE", "0") == "1"
    res = run_bass_kernel_spmd(nc, in_maps, core_ids=list(range(N_CORES)),
                               trace=trace)
    _LAST_RESULTS = res

    S = np.stack([np.asarray(res.results[c]["out"]) for c in range(N_CORES)])
    # S[c][p, g] = -ap(row c*1024 + g*128 + p)
    ap = -S.transpose(0, 2, 1).reshape(-1).astype(np.float64)
    apm = ap.mean()
    return np.array([1.0 - apm, apm], dtype=np.float32)
